# revision 8
# baseline (speedup 1.0000x reference)
"""Cross-modal attention (B=4, C=512, L=2048, H=8, D=64) on 8 TRN2 NeuronCores.

Sharding: core c handles batch b = c//2 and query-half q = c%2 (1024 queries).
K/V are computed from the full ecg[b] on both cores of a pair (duplicated, no
collectives needed).  All compute in float32r matmuls (full PE rate), fp32
accumulation.

Layout trick: inputs ppg/ecg arrive as (C, L) = x^T, which is exactly the
lhsT/rhs layouts the TensorEngine wants, and the output is produced directly
in (C, L) layout — the kernel contains no runtime transposes.  Weights are
transposed once on the host.

Per-core pipeline:
  phase 1: qT = Wq @ x^T  (C x Lq),  kT = Wk @ y^T (C x L),
           v = y @ Wv^T (L x C, head-strided with a ones column appended)
  phase 2: per (head-pair, q-block): scores^T = kT_h^T qT_h (keys x q) into
           4-bank PSUM groups -> exp on ACT -> ctx^T accumulation
           (v_aug^T @ exp) with softmax denominator in row 64 ->
           reciprocal + partition_broadcast + DVE multiply.
  phase 3: out^T = Wo @ ctx^T + bo + x^T, DMA out.
"""

import numpy as np

B = 4
C = 512
L = 2048
H = 8
D = 64
LQ = 1024          # queries per core
P = 128
NCB = C // P       # 4 c-blocks
NQB = LQ // 512    # 2 query blocks of 512
NKB = L // P       # 16 key blocks of 128
KB_GROUP = 2       # key blocks per exp group (4 psum banks = 2 kb x 2 heads)

_CACHED = {}


def _build():
    import concourse.tile as tile
    from concourse import bacc, mybir

    F32 = mybir.dt.float32
    F32R = mybir.dt.float32r
    EXP = mybir.ActivationFunctionType.Exp

    nc = bacc.Bacc("TRN2", target_bir_lowering=False, debug=False)

    ppg_q = nc.dram_tensor("ppg_q", (C, LQ), F32, kind="ExternalInput").ap()
    ecg_b = nc.dram_tensor("ecg_b", (C, L), F32, kind="ExternalInput").ap()
    wqt = nc.dram_tensor("wqt", (C, C), F32, kind="ExternalInput").ap()
    wkt = nc.dram_tensor("wkt", (C, C), F32, kind="ExternalInput").ap()
    wvt = nc.dram_tensor("wvt", (C, C), F32, kind="ExternalInput").ap()
    wot = nc.dram_tensor("wot", (C, C), F32, kind="ExternalInput").ap()
    bq = nc.dram_tensor("bq", (C,), F32, kind="ExternalInput").ap()
    bk = nc.dram_tensor("bk", (C,), F32, kind="ExternalInput").ap()
    bv = nc.dram_tensor("bv", (C,), F32, kind="ExternalInput").ap()
    bo = nc.dram_tensor("bo", (C,), F32, kind="ExternalInput").ap()
    outp = nc.dram_tensor("outp", (C, LQ), F32, kind="ExternalOutput").ap()

    with tile.TileContext(nc) as tc:
        with tc.tile_pool(name="persist", bufs=1) as persist:
            # ---- persistent constants ----
            wot64 = persist.tile([64, H, C], F32R)   # Wo^T rows regrouped by head
            nc.sync.dma_start(
                wot64[:], wot.rearrange("(h d) o -> d h o", d=64).bitcast(F32R))
            bq_t = persist.tile([P, NCB], F32)
            bk_t = persist.tile([P, NCB], F32)
            bo_t = persist.tile([P, NCB], F32)
            nc.sync.dma_start(bq_t[:], bq.rearrange("(s p) -> p s", p=P))
            nc.sync.dma_start(bk_t[:], bk.rearrange("(s p) -> p s", p=P))
            nc.sync.dma_start(bo_t[:], bo.rearrange("(s p) -> p s", p=P))
            bv_row = persist.tile([1, C], F32R)
            nc.sync.dma_start(bv_row[0:1, :], bv[None, :].bitcast(F32R))
            ones_f = persist.tile([1, P], F32)
            nc.vector.memset(ones_f[:], 1.0)
            ones_t = persist.tile([1, P], F32R)
            nc.vector.tensor_copy(out=ones_t[:], in_=ones_f[:])
            ones_col = persist.tile([P, 1], F32)
            nc.vector.memset(ones_col[:], 1.0)

            # ---- persistent activations ----
            # ppg kept once in F32R; DMA is a bitcast so the bits are exact
            # fp32 — the residual add reads it bitcast back to F32.
            ppg_r = persist.tile([P, NCB, LQ], F32R)
            nc.sync.dma_start(
                ppg_r[:], ppg_q.rearrange("(s p) l -> p s l", p=P).bitcast(F32R))
            qT = persist.tile([P, NCB, LQ], F32R)
            kT = persist.tile([P, NCB, L], F32R)
            v = persist.tile([P, NKB, H, D + 1], F32R)   # v with ones column
            ctxT = persist.tile([64, H, LQ], F32R)
            # ones column of v_aug
            nc.vector.tensor_copy(
                out=v[:, :, :, D:D + 1],
                in_=ones_col[:, None, None, :].to_broadcast((P, NKB, H, 1)))

            # ================= phase 1: projections =================
            with (
                tc.tile_pool(name="ph1_in", bufs=1) as ph1_in,
                tc.tile_pool(name="ph1_ps", bufs=4, space="PSUM") as ph1_ps,
            ):
                ecg_r = ph1_in.tile([P, NCB, L], F32R)
                nc.sync.dma_start(
                    ecg_r[:], ecg_b.rearrange("(s p) l -> p s l", p=P).bitcast(F32R))
                wqt_t = ph1_in.tile([P, NCB, C], F32R)
                wkt_t = ph1_in.tile([P, NCB, C], F32R)
                wvt_t = ph1_in.tile([P, NCB, C], F32R)
                nc.sync.dma_start(
                    wqt_t[:], wqt.rearrange("(s p) o -> p s o", p=P).bitcast(F32R))
                nc.sync.dma_start(
                    wkt_t[:], wkt.rearrange("(s p) o -> p s o", p=P).bitcast(F32R))
                nc.sync.dma_start(
                    wvt_t[:], wvt.rearrange("(s p) o -> p s o", p=P).bitcast(F32R))

                # v = y @ Wv^T + bv   (L x C), head-strided into v_aug
                for lb in range(NKB):
                    ps_v = ph1_ps.tile([P, 512], F32, tag="ps1")
                    nc.tensor.matmul(ps_v[:], ones_t[0:1, :], bv_row[0:1, :],
                                     start=True, stop=False)
                    for s in range(NCB):
                        nc.tensor.matmul(
                            ps_v[:], ecg_r[:, s, lb * P:(lb + 1) * P],
                            wvt_t[:, s, :], start=False, stop=(s == NCB - 1))
                    nc.vector.tensor_copy(
                        out=v[:, lb, :, 0:D],
                        in_=ps_v[:].rearrange("p (h d) -> p h d", d=D))

                # kT = Wk @ y^T + bk   (C x L)
                for cb in range(NCB):
                    for kb in range(L // 512):
                        ps_k = ph1_ps.tile([P, 512], F32, tag="ps1")
                        for s in range(NCB):
                            nc.tensor.matmul(
                                ps_k[:], wkt_t[:, s, cb * P:(cb + 1) * P],
                                ecg_r[:, s, kb * 512:(kb + 1) * 512],
                                start=(s == 0), stop=(s == NCB - 1))
                        nc.vector.tensor_scalar_add(
                            kT[:, cb, kb * 512:(kb + 1) * 512], ps_k[:],
                            bk_t[:, cb:cb + 1])

                # qT = Wq @ x^T + bq   (C x Lq)
                for cb in range(NCB):
                    for qb in range(NQB):
                        ps_q = ph1_ps.tile([P, 512], F32, tag="ps1")
                        for s in range(NCB):
                            nc.tensor.matmul(
                                ps_q[:], wqt_t[:, s, cb * P:(cb + 1) * P],
                                ppg_r[:, s, qb * 512:(qb + 1) * 512],
                                start=(s == 0), stop=(s == NCB - 1))
                        nc.vector.tensor_scalar_add(
                            qT[:, cb, qb * 512:(qb + 1) * 512], ps_q[:],
                            bq_t[:, cb:cb + 1])

            # ================= phase 2: attention =================
            with (
                tc.tile_pool(name="ps_s", bufs=1, space="PSUM") as ps_s,
                tc.tile_pool(name="ps_c", bufs=2, space="PSUM") as ps_c,
                tc.tile_pool(name="exp_pool", bufs=2) as exp_pool,
                tc.tile_pool(name="sm_pool", bufs=2) as sm_pool,
            ):
                for pair in range(H // 2):
                    for qb in range(NQB):
                        qsl = slice(qb * 512, (qb + 1) * 512)
                        pc0 = ps_c.tile([P, 512], F32)
                        pc1 = ps_c.tile([P, 512], F32)
                        pcs = (pc0, pc1)
                        for g in range(NKB // KB_GROUP):
                            st = ps_s.tile([P, KB_GROUP, 2, 512], F32)
                            for kbl in range(KB_GROUP):
                                kb = g * KB_GROUP + kbl
                                for hl in range(2):
                                    nc.tensor.matmul(
                                        st[:, kbl, hl, :],
                                        kT[64 * hl:64 * hl + 64, pair,
                                           kb * P:(kb + 1) * P],
                                        qT[64 * hl:64 * hl + 64, pair, qsl],
                                        start=True, stop=True)
                            et = exp_pool.tile([P, KB_GROUP, 2, 512], F32R)
                            nc.scalar.activation(et[:], st[:], EXP, scale=0.125)
                            for kbl in range(KB_GROUP):
                                kb = g * KB_GROUP + kbl
                                for hl in range(2):
                                    nc.tensor.matmul(
                                        pcs[hl][0:D + 1, :],
                                        v[:, kb, 2 * pair + hl, :],
                                        et[:, kbl, hl, :],
                                        start=(kb == 0), stop=(kb == NKB - 1))
                        for hl in range(2):
                            h = 2 * pair + hl
                            recip = sm_pool.tile([1, 512], F32)
                            nc.vector.reciprocal(recip[0:1, :],
                                                 pcs[hl][D:D + 1, :])
                            rbc = sm_pool.tile([64, 512], F32)
                            nc.gpsimd.partition_broadcast(rbc[:], recip[0:1, :],
                                                          channels=64)
                            nc.vector.tensor_mul(
                                out=ctxT[:, h, qsl], in0=pcs[hl][0:D, :],
                                in1=rbc[:])

                # ============= phase 3: output projection =============
                with tc.tile_pool(name="out_sb", bufs=3) as out_sb:
                    for cb in range(NCB):
                        for qb in range(NQB):
                            qsl = slice(qb * 512, (qb + 1) * 512)
                            po = ps_c.tile([P, 512], F32, tag="pc0")
                            for h in range(H):
                                nc.tensor.matmul(
                                    po[:], wot64[:, h, cb * P:(cb + 1) * P],
                                    ctxT[:, h, qsl],
                                    start=(h == 0), stop=(h == H - 1))
                            ot = out_sb.tile([P, 512], F32)
                            nc.vector.tensor_scalar_add(ot[:], po[:],
                                                        bo_t[:, cb:cb + 1])
                            nc.vector.tensor_add(ot[:], ot[:],
                                                 ppg_r[:, cb, qsl].bitcast(F32))
                            nc.sync.dma_start(
                                outp.rearrange("(s p) l -> p s l", p=P)[:, cb, qsl],
                                ot[:])
    nc.compile()
    return nc


def _get_nc():
    if "nc" not in _CACHED:
        _CACHED["nc"] = _build()
    return _CACHED["nc"]


def kernel(ppg, ecg, Wq, bq, Wk, bk, Wv, bv, Wo, bo):
    from concourse.bass_utils import run_bass_kernel_spmd

    nc = _get_nc()
    f = np.float32
    wqt = np.ascontiguousarray(np.asarray(Wq, f).T)
    wkt = np.ascontiguousarray(np.asarray(Wk, f).T)
    wvt = np.ascontiguousarray(np.asarray(Wv, f).T)
    wot = np.ascontiguousarray(np.asarray(Wo, f).T)
    ppg = np.asarray(ppg, f)
    ecg = np.asarray(ecg, f)
    in_maps = []
    for c in range(8):
        b, half = c // 2, c % 2
        in_maps.append({
            "ppg_q": np.ascontiguousarray(ppg[b][:, half * LQ:(half + 1) * LQ]),
            "ecg_b": np.ascontiguousarray(ecg[b]),
            "wqt": wqt, "wkt": wkt, "wvt": wvt, "wot": wot,
            "bq": np.asarray(bq, f), "bk": np.asarray(bk, f),
            "bv": np.asarray(bv, f), "bo": np.asarray(bo, f),
        })
    _CACHED["last_in_maps"] = in_maps
    res = run_bass_kernel_spmd(nc, in_maps, core_ids=list(range(8)))
    out = np.empty((B, C, L), f)
    for c, r in enumerate(res.results):
        b, half = c // 2, c % 2
        out[b][:, half * LQ:(half + 1) * LQ] = r["outp"]
    return out


# revision 15
# speedup vs baseline: 1.0824x; 1.0824x over previous
"""Cross-modal attention (B=4, C=512, L=2048, H=8, D=64) on 8 TRN2 NeuronCores.

Sharding: core c handles batch b = c//2 and query-half q = c%2 (1024 queries).
K/V are computed from the full ecg[b] on both cores of a pair (duplicated, no
collectives needed).  Matmuls run in bf16 (full PE rate, warms the HAM clock
gate); accumulation is fp32 in PSUM, softmax/normalization/residual in fp32.

Layout trick: inputs ppg/ecg arrive as (C, L) = x^T, which is exactly the
lhsT/rhs layouts the TensorEngine wants, and the output is produced directly
in (C, L) layout — the kernel contains no runtime transposes.  Weights are
transposed once on the host.

Per-core pipeline:
  phase 1: qT = Wq @ x^T  (C x Lq),  kT = Wk @ y^T (C x L),
           v = y @ Wv^T (L x C, head-strided with a ones column appended)
  phase 2: per head-pair: per key-block: scores^T (keys x q) for both heads
           into a 4-bank PSUM group -> exp on ACT (bf16 out) -> ctx^T
           accumulation (v_aug^T @ exp) with the softmax denominator landing
           in row 64 -> reciprocal + partition_broadcast + DVE multiply.
  phase 3: out^T = Wo @ ctx^T + bo + x^T, DMA out.
"""

import os
import numpy as np

B = 4
C = 512
L = 2048
H = 8
D = 64
LQ = 1024          # queries per core = matmul moving free dim (bf16 max 1024)
P = 128
NCB = C // P       # 4 c-blocks
NKB = L // P       # 16 key blocks of 128

_CACHED = {}


def _build():
    import concourse.tile as tile
    from concourse import bacc, mybir

    F32 = mybir.dt.float32
    CDT = mybir.dt.bfloat16
    EXP = mybir.ActivationFunctionType.Exp

    nc = bacc.Bacc("TRN2", target_bir_lowering=False, debug=False)

    ppg_q = nc.dram_tensor("ppg_q", (C, LQ), F32, kind="ExternalInput").ap()
    ecg_b = nc.dram_tensor("ecg_b", (C, L), F32, kind="ExternalInput").ap()
    wqt = nc.dram_tensor("wqt", (C, C), F32, kind="ExternalInput").ap()
    wkt = nc.dram_tensor("wkt", (C, C), F32, kind="ExternalInput").ap()
    wvt = nc.dram_tensor("wvt", (C, C), F32, kind="ExternalInput").ap()
    wot = nc.dram_tensor("wot", (C, C), F32, kind="ExternalInput").ap()
    bq = nc.dram_tensor("bq", (C,), F32, kind="ExternalInput").ap()
    bk = nc.dram_tensor("bk", (C,), F32, kind="ExternalInput").ap()
    bv = nc.dram_tensor("bv", (C,), F32, kind="ExternalInput").ap()
    bo = nc.dram_tensor("bo", (C,), F32, kind="ExternalInput").ap()
    outp = nc.dram_tensor("outp", (C, LQ), F32, kind="ExternalOutput").ap()
    dbg = {}
    if os.environ.get("KDBG"):
        dbg["qT"] = nc.dram_tensor("d_qT", (P, NCB, LQ), F32,
                                   kind="ExternalOutput").ap()
        dbg["kT"] = nc.dram_tensor("d_kT", (P, NCB, L), F32,
                                   kind="ExternalOutput").ap()
        dbg["v"] = nc.dram_tensor("d_v", (P, NKB, H, D + 1), F32,
                                  kind="ExternalOutput").ap()
        dbg["ctxT"] = nc.dram_tensor("d_ctxT", (64, H, LQ), F32,
                                     kind="ExternalOutput").ap()

    with tile.TileContext(nc) as tc:
        with tc.tile_pool(name="persist", bufs=1) as persist:
            # ---- persistent constants ----
            wot64 = persist.tile([64, H, C], CDT)   # Wo^T rows regrouped by head
            nc.gpsimd.dma_start(wot64[:], wot.rearrange("(h d) o -> d h o", d=64))
            bq_t = persist.tile([P, NCB], F32)
            bk_t = persist.tile([P, NCB], F32)
            bo_t = persist.tile([P, NCB], F32)
            nc.sync.dma_start(bq_t[:], bq.rearrange("(s p) -> p s", p=P))
            nc.sync.dma_start(bk_t[:], bk.rearrange("(s p) -> p s", p=P))
            nc.sync.dma_start(bo_t[:], bo.rearrange("(s p) -> p s", p=P))
            bv_row = persist.tile([1, C], CDT)
            nc.gpsimd.dma_start(bv_row[0:1, :], bv[None, :])
            ones_t = persist.tile([1, P], CDT)
            nc.vector.memset(ones_t[:], 1.0)
            ones_col = persist.tile([P, 1], F32)
            nc.vector.memset(ones_col[:], 1.0)

            # ---- persistent activations ----
            ppg_f = persist.tile([P, NCB, LQ], F32)      # exact residual
            nc.sync.dma_start(ppg_f[:], ppg_q.rearrange("(s p) l -> p s l", p=P))
            ppg_c = persist.tile([P, NCB, LQ], CDT)      # matmul operand
            nc.gpsimd.dma_start(ppg_c[:], ppg_q.rearrange("(s p) l -> p s l", p=P))
            qT = persist.tile([P, NCB, LQ], CDT)
            kT = persist.tile([P, NCB, L], CDT)
            v = persist.tile([P, NKB, H, D + 1], CDT)    # v with ones column
            ctxT = persist.tile([64, H, LQ], CDT)
            nc.vector.tensor_copy(
                out=v[:, :, :, D:D + 1],
                in_=ones_col[:, None, None, :].to_broadcast((P, NKB, H, 1)))

            # ================= phase 1: projections =================
            with (
                tc.tile_pool(name="ph1_in", bufs=1) as ph1_in,
                tc.tile_pool(name="ph1_ps", bufs=1, space="PSUM") as ph1_ps,
            ):
                ecg_r = ph1_in.tile([P, NCB, L], CDT)
                nc.gpsimd.dma_start(
                    ecg_r[:], ecg_b.rearrange("(s p) l -> p s l", p=P))
                wqt_t = ph1_in.tile([P, NCB, C], CDT)
                wkt_t = ph1_in.tile([P, NCB, C], CDT)
                wvt_t = ph1_in.tile([P, NCB, C], CDT)
                nc.gpsimd.dma_start(wqt_t[:], wqt.rearrange("(s p) o -> p s o", p=P))
                nc.gpsimd.dma_start(wkt_t[:], wkt.rearrange("(s p) o -> p s o", p=P))
                nc.gpsimd.dma_start(wvt_t[:], wvt.rearrange("(s p) o -> p s o", p=P))

                # v = y @ Wv^T + bv   (L x C), head-strided into v_aug
                for lb in range(NKB):
                    ps_v = ph1_ps.tile([P, 512], F32, tag="pv", bufs=2)
                    nc.tensor.matmul(ps_v[:], ones_t[0:1, :], bv_row[0:1, :],
                                     start=True, stop=False)
                    for s in range(NCB):
                        nc.tensor.matmul(
                            ps_v[:], ecg_r[:, s, lb * P:(lb + 1) * P],
                            wvt_t[:, s, :], start=False, stop=(s == NCB - 1))
                    nc.vector.tensor_copy(
                        out=v[:, lb, :, 0:D],
                        in_=ps_v[:].rearrange("p (h d) -> p h d", d=D))

                # kT = Wk @ y^T + bk   (C x L)
                for cb in range(NCB):
                    for kb in range(L // 512):
                        ps_k = ph1_ps.tile([P, 512], F32, tag="pk", bufs=4)
                        for s in range(NCB):
                            nc.tensor.matmul(
                                ps_k[:], wkt_t[:, s, cb * P:(cb + 1) * P],
                                ecg_r[:, s, kb * 512:(kb + 1) * 512],
                                start=(s == 0), stop=(s == NCB - 1))
                        nc.vector.tensor_scalar_add(
                            kT[:, cb, kb * 512:(kb + 1) * 512], ps_k[:],
                            bk_t[:, cb:cb + 1])

                # qT = Wq @ x^T + bq   (C x Lq)
                for cb in range(NCB):
                    for qb in range(LQ // 512):
                        ps_q = ph1_ps.tile([P, 512], F32, tag="pk", bufs=4)
                        for s in range(NCB):
                            nc.tensor.matmul(
                                ps_q[:], wqt_t[:, s, cb * P:(cb + 1) * P],
                                ppg_c[:, s, qb * 512:(qb + 1) * 512],
                                start=(s == 0), stop=(s == NCB - 1))
                        nc.vector.tensor_scalar_add(
                            qT[:, cb, qb * 512:(qb + 1) * 512], ps_q[:],
                            bq_t[:, cb:cb + 1])

            # ================= phase 2: attention =================
            with (
                tc.tile_pool(name="ps_s", bufs=1, space="PSUM") as ps_s,
                tc.tile_pool(name="ps_c", bufs=1, space="PSUM") as ps_c,
                tc.tile_pool(name="exp_pool", bufs=2) as exp_pool,
                tc.tile_pool(name="sm_pool", bufs=2) as sm_pool,
            ):
                for pair in range(H // 2):
                    for qb in range(LQ // 512):
                        qsl = slice(qb * 512, (qb + 1) * 512)
                        pc0 = ps_c.tile([P, 512], F32, tag="pc0", bufs=2)
                        pc1 = ps_c.tile([P, 512], F32, tag="pc1", bufs=2)
                        pcs = (pc0, pc1)
                        for g in range(NKB // 2):
                            st = ps_s.tile([P, 2, 2, 512], F32)
                            for kbl in range(2):
                                kb = g * 2 + kbl
                                for hl in range(2):
                                    nc.tensor.matmul(
                                        st[:, kbl, hl, :],
                                        kT[64 * hl:64 * hl + 64, pair,
                                           kb * P:(kb + 1) * P],
                                        qT[64 * hl:64 * hl + 64, pair, qsl],
                                        start=True, stop=True)
                            et = exp_pool.tile([P, 2, 2, 512], CDT)
                            nc.scalar.activation(et[:], st[:], EXP, scale=0.125)
                            for kbl in range(2):
                                kb = g * 2 + kbl
                                for hl in range(2):
                                    nc.tensor.matmul(
                                        pcs[hl][0:D + 1, :],
                                        v[:, kb, 2 * pair + hl, :],
                                        et[:, kbl, hl, :],
                                        start=(kb == 0), stop=(kb == NKB - 1))
                        for hl in range(2):
                            h = 2 * pair + hl
                            den = sm_pool.tile([1, 512], F32)
                            nc.vector.tensor_copy(out=den[0:1, :],
                                                  in_=pcs[hl][D:D + 1, :])
                            recip = sm_pool.tile([1, 512], F32)
                            nc.vector.reciprocal_approx_fast(
                                out=recip[0:1, :], in_=den[0:1, :])
                            rbc = sm_pool.tile([64, 512], F32)
                            nc.gpsimd.partition_broadcast(rbc[:], recip[0:1, :],
                                                          channels=64)
                            nc.vector.tensor_mul(
                                out=ctxT[:, h, qsl], in0=pcs[hl][0:D, :],
                                in1=rbc[:])

                # ============= phase 3: output projection =============
                with tc.tile_pool(name="out_sb", bufs=3) as out_sb:
                    for cb in range(NCB):
                        for qb in range(LQ // 512):
                            qsl = slice(qb * 512, (qb + 1) * 512)
                            po = ps_c.tile([P, 512], F32, tag="pc0", bufs=2)
                            for h in range(H):
                                nc.tensor.matmul(
                                    po[:], wot64[:, h, cb * P:(cb + 1) * P],
                                    ctxT[:, h, qsl],
                                    start=(h == 0), stop=(h == H - 1))
                            ot = out_sb.tile([P, 512], F32)
                            nc.vector.tensor_scalar_add(ot[:], po[:],
                                                        bo_t[:, cb:cb + 1])
                            nc.vector.tensor_add(ot[:], ot[:],
                                                 ppg_f[:, cb, qsl])
                            nc.sync.dma_start(
                                outp.rearrange("(s p) l -> p s l",
                                               p=P)[:, cb, qsl],
                                ot[:])
            if dbg:
                for name, src in (("qT", qT), ("kT", kT), ("v", v),
                                  ("ctxT", ctxT)):
                    nc.gpsimd.dma_start(dbg[name], src[:])
    nc.compile()
    return nc


def _get_nc():
    if "nc" not in _CACHED:
        _CACHED["nc"] = _build()
    return _CACHED["nc"]


def kernel(ppg, ecg, Wq, bq, Wk, bk, Wv, bv, Wo, bo):
    from concourse.bass_utils import run_bass_kernel_spmd

    nc = _get_nc()
    f = np.float32
    wqt = np.ascontiguousarray(np.asarray(Wq, f).T)
    wkt = np.ascontiguousarray(np.asarray(Wk, f).T)
    wvt = np.ascontiguousarray(np.asarray(Wv, f).T)
    wot = np.ascontiguousarray(np.asarray(Wo, f).T)
    ppg = np.asarray(ppg, f)
    ecg = np.asarray(ecg, f)
    in_maps = []
    for c in range(8):
        b, half = c // 2, c % 2
        in_maps.append({
            "ppg_q": np.ascontiguousarray(ppg[b][:, half * LQ:(half + 1) * LQ]),
            "ecg_b": np.ascontiguousarray(ecg[b]),
            "wqt": wqt, "wkt": wkt, "wvt": wvt, "wot": wot,
            "bq": np.asarray(bq, f), "bk": np.asarray(bk, f),
            "bv": np.asarray(bv, f), "bo": np.asarray(bo, f),
        })
    _CACHED["last_in_maps"] = in_maps
    res = run_bass_kernel_spmd(nc, in_maps, core_ids=list(range(8)))
    out = np.empty((B, C, L), f)
    for c, r in enumerate(res.results):
        b, half = c // 2, c % 2
        out[b][:, half * LQ:(half + 1) * LQ] = r["outp"]
    return out


# revision 16
# speedup vs baseline: 1.5469x; 1.4292x over previous
"""Cross-modal attention (B=4, C=512, L=2048, H=8, D=64) on 8 TRN2 NeuronCores.

Sharding: core c handles batch b = c//2 and query-half q = c%2 (1024 queries).
K/V are computed from the full ecg[b] on both cores of a pair (duplicated, no
collectives needed).  Matmuls run in bf16 (full PE rate, warms the HAM clock
gate); accumulation is fp32 in PSUM, softmax/normalization/residual in fp32.

Layout trick: inputs ppg/ecg arrive as (C, L) = x^T, which is exactly the
lhsT/rhs layouts the TensorEngine wants, and the output is produced directly
in (C, L) layout — the kernel contains no runtime transposes.  Weights are
transposed once on the host.

Per-core pipeline:
  phase 1: qT = Wq @ x^T  (C x Lq),  kT = Wk @ y^T (C x L),
           v = y @ Wv^T (L x C, head-strided with a ones column appended)
  phase 2: per head-pair: per key-block: scores^T (keys x q) for both heads
           into a 4-bank PSUM group -> exp on ACT (bf16 out) -> ctx^T
           accumulation (v_aug^T @ exp) with the softmax denominator landing
           in row 64 -> reciprocal + partition_broadcast + DVE multiply.
  phase 3: out^T = Wo @ ctx^T + bo + x^T, DMA out.
"""

import os
import numpy as np

B = 4
C = 512
L = 2048
H = 8
D = 64
LQ = 1024          # queries per core = matmul moving free dim (bf16 max 1024)
P = 128
NCB = C // P       # 4 c-blocks
NKB = L // P       # 16 key blocks of 128

_CACHED = {}


def _build():
    import concourse.tile as tile
    from concourse import bacc, mybir

    F32 = mybir.dt.float32
    CDT = mybir.dt.bfloat16
    EXP = mybir.ActivationFunctionType.Exp

    nc = bacc.Bacc("TRN2", target_bir_lowering=False, debug=False)

    ppg_q = nc.dram_tensor("ppg_q", (C, LQ), F32, kind="ExternalInput").ap()
    ecg_b = nc.dram_tensor("ecg_b", (C, L), F32, kind="ExternalInput").ap()
    wqt = nc.dram_tensor("wqt", (C, C), F32, kind="ExternalInput").ap()
    wkt = nc.dram_tensor("wkt", (C, C), F32, kind="ExternalInput").ap()
    wvt = nc.dram_tensor("wvt", (C, C), F32, kind="ExternalInput").ap()
    wot = nc.dram_tensor("wot", (C, C), F32, kind="ExternalInput").ap()
    bq = nc.dram_tensor("bq", (C,), F32, kind="ExternalInput").ap()
    bk = nc.dram_tensor("bk", (C,), F32, kind="ExternalInput").ap()
    bv = nc.dram_tensor("bv", (C,), F32, kind="ExternalInput").ap()
    bo = nc.dram_tensor("bo", (C,), F32, kind="ExternalInput").ap()
    outp = nc.dram_tensor("outp", (C, LQ), F32, kind="ExternalOutput").ap()
    dbg = {}
    if os.environ.get("KDBG"):
        dbg["qT"] = nc.dram_tensor("d_qT", (P, NCB, LQ), F32,
                                   kind="ExternalOutput").ap()
        dbg["kT"] = nc.dram_tensor("d_kT", (P, NCB, L), F32,
                                   kind="ExternalOutput").ap()
        dbg["v"] = nc.dram_tensor("d_v", (P, NKB, H, D + 1), F32,
                                  kind="ExternalOutput").ap()
        dbg["ctxT"] = nc.dram_tensor("d_ctxT", (64, H, LQ), F32,
                                     kind="ExternalOutput").ap()

    with tile.TileContext(nc) as tc:
        with tc.tile_pool(name="persist", bufs=1) as persist:
            # ---- persistent constants ----
            wot64 = persist.tile([64, H, C], CDT)   # Wo^T rows regrouped by head
            nc.gpsimd.dma_start(wot64[:], wot.rearrange("(h d) o -> d h o", d=64))
            bq_t = persist.tile([P, NCB], F32)
            bk_t = persist.tile([P, NCB], F32)
            bo_t = persist.tile([P, NCB], F32)
            nc.sync.dma_start(bq_t[:], bq.rearrange("(s p) -> p s", p=P))
            nc.sync.dma_start(bk_t[:], bk.rearrange("(s p) -> p s", p=P))
            nc.sync.dma_start(bo_t[:], bo.rearrange("(s p) -> p s", p=P))
            bv_row = persist.tile([1, C], CDT)
            nc.gpsimd.dma_start(bv_row[0:1, :], bv[None, :])
            ones_t = persist.tile([1, P], CDT)
            nc.vector.memset(ones_t[:], 1.0)
            ones_col = persist.tile([P, 1], F32)
            nc.vector.memset(ones_col[:], 1.0)

            # ---- persistent activations ----
            ppg_f = persist.tile([P, NCB, LQ], F32)      # exact residual
            nc.sync.dma_start(ppg_f[:], ppg_q.rearrange("(s p) l -> p s l", p=P))
            ppg_c = persist.tile([P, NCB, LQ], CDT)      # matmul operand
            nc.gpsimd.dma_start(ppg_c[:], ppg_q.rearrange("(s p) l -> p s l", p=P))
            qT = persist.tile([P, NCB, LQ], CDT)
            kT = persist.tile([P, NCB, L], CDT)
            v = persist.tile([P, NKB, H, D + 1], CDT)    # v with ones column
            ctxT = persist.tile([64, H, LQ], CDT)
            nc.vector.tensor_copy(
                out=v[:, :, :, D:D + 1],
                in_=ones_col[:, None, None, :].to_broadcast((P, NKB, H, 1)))

            # ================= phase 1: projections =================
            with (
                tc.tile_pool(name="ph1_in", bufs=1) as ph1_in,
                tc.tile_pool(name="ph1_ps", bufs=1, space="PSUM") as ph1_ps,
            ):
                ecg_r = ph1_in.tile([P, NCB, L], CDT)
                nc.gpsimd.dma_start(
                    ecg_r[:], ecg_b.rearrange("(s p) l -> p s l", p=P))
                wqt_t = ph1_in.tile([P, NCB, C], CDT)
                wkt_t = ph1_in.tile([P, NCB, C], CDT)
                wvt_t = ph1_in.tile([P, NCB, C], CDT)
                nc.gpsimd.dma_start(wqt_t[:], wqt.rearrange("(s p) o -> p s o", p=P))
                nc.gpsimd.dma_start(wkt_t[:], wkt.rearrange("(s p) o -> p s o", p=P))
                nc.gpsimd.dma_start(wvt_t[:], wvt.rearrange("(s p) o -> p s o", p=P))

                # v = y @ Wv^T + bv   (L x C), head-strided into v_aug
                for lb in range(NKB):
                    ps_v = ph1_ps.tile([P, 512], F32, tag="pv", bufs=2)
                    nc.tensor.matmul(ps_v[:], ones_t[0:1, :], bv_row[0:1, :],
                                     start=True, stop=False)
                    for s in range(NCB):
                        nc.tensor.matmul(
                            ps_v[:], ecg_r[:, s, lb * P:(lb + 1) * P],
                            wvt_t[:, s, :], start=False, stop=(s == NCB - 1))
                    nc.vector.tensor_copy(
                        out=v[:, lb, :, 0:D],
                        in_=ps_v[:].rearrange("p (h d) -> p h d", d=D))

                # kT = Wk @ y^T + bk   (C x L)
                for cb in range(NCB):
                    for kb in range(L // 512):
                        ps_k = ph1_ps.tile([P, 512], F32, tag="pk", bufs=4)
                        for s in range(NCB):
                            nc.tensor.matmul(
                                ps_k[:], wkt_t[:, s, cb * P:(cb + 1) * P],
                                ecg_r[:, s, kb * 512:(kb + 1) * 512],
                                start=(s == 0), stop=(s == NCB - 1))
                        nc.vector.tensor_scalar_add(
                            kT[:, cb, kb * 512:(kb + 1) * 512], ps_k[:],
                            bk_t[:, cb:cb + 1])

                # qT = Wq @ x^T + bq   (C x Lq)
                for cb in range(NCB):
                    for qb in range(LQ // 512):
                        ps_q = ph1_ps.tile([P, 512], F32, tag="pk", bufs=4)
                        for s in range(NCB):
                            nc.tensor.matmul(
                                ps_q[:], wqt_t[:, s, cb * P:(cb + 1) * P],
                                ppg_c[:, s, qb * 512:(qb + 1) * 512],
                                start=(s == 0), stop=(s == NCB - 1))
                        nc.vector.tensor_scalar_add(
                            qT[:, cb, qb * 512:(qb + 1) * 512], ps_q[:],
                            bq_t[:, cb:cb + 1])

            # ================= phase 2: attention =================
            with (
                tc.tile_pool(name="ps_s", bufs=1, space="PSUM") as ps_s,
                tc.tile_pool(name="ps_c", bufs=1, space="PSUM") as ps_c,
                tc.tile_pool(name="exp_pool", bufs=2) as exp_pool,
                tc.tile_pool(name="sm_pool", bufs=2) as sm_pool,
            ):
                for pair in range(H // 2):
                    for qb in range(LQ // 512):
                        qsl = slice(qb * 512, (qb + 1) * 512)
                        pc0 = ps_c.tile([P, 512], F32, tag="pc0", bufs=1)
                        pc1 = ps_c.tile([P, 512], F32, tag="pc1", bufs=1)
                        pcs = (pc0, pc1)
                        for kb in range(NKB):
                            st = ps_s.tile([P, 2, 512], F32, bufs=3)
                            for hl in range(2):
                                nc.tensor.matmul(
                                    st[:, hl, :],
                                    kT[64 * hl:64 * hl + 64, pair,
                                       kb * P:(kb + 1) * P],
                                    qT[64 * hl:64 * hl + 64, pair, qsl],
                                    start=True, stop=True)
                            et = exp_pool.tile([P, 2, 512], CDT, bufs=4)
                            nc.scalar.activation(et[:], st[:], EXP, scale=0.125)
                            for hl in range(2):
                                nc.tensor.matmul(
                                    pcs[hl][0:D + 1, :],
                                    v[:, kb, 2 * pair + hl, :],
                                    et[:, hl, :],
                                    start=(kb == 0), stop=(kb == NKB - 1))
                        for hl in range(2):
                            h = 2 * pair + hl
                            den = sm_pool.tile([1, 512], F32)
                            nc.vector.tensor_copy(out=den[0:1, :],
                                                  in_=pcs[hl][D:D + 1, :])
                            recip = sm_pool.tile([1, 512], F32)
                            nc.vector.reciprocal_approx_fast(
                                out=recip[0:1, :], in_=den[0:1, :])
                            rbc = sm_pool.tile([64, 512], F32)
                            nc.gpsimd.partition_broadcast(rbc[:], recip[0:1, :],
                                                          channels=64)
                            nc.vector.tensor_mul(
                                out=ctxT[:, h, qsl], in0=pcs[hl][0:D, :],
                                in1=rbc[:])

                # ============= phase 3: output projection =============
                with tc.tile_pool(name="out_sb", bufs=3) as out_sb:
                    for cb in range(NCB):
                        for qb in range(LQ // 512):
                            qsl = slice(qb * 512, (qb + 1) * 512)
                            po = ps_c.tile([P, 512], F32, tag="pc0", bufs=1)
                            for h in range(H):
                                nc.tensor.matmul(
                                    po[:], wot64[:, h, cb * P:(cb + 1) * P],
                                    ctxT[:, h, qsl],
                                    start=(h == 0), stop=(h == H - 1))
                            ot = out_sb.tile([P, 512], F32)
                            nc.vector.tensor_scalar_add(ot[:], po[:],
                                                        bo_t[:, cb:cb + 1])
                            nc.vector.tensor_add(ot[:], ot[:],
                                                 ppg_f[:, cb, qsl])
                            nc.sync.dma_start(
                                outp.rearrange("(s p) l -> p s l",
                                               p=P)[:, cb, qsl],
                                ot[:])
            if dbg:
                for name, src in (("qT", qT), ("kT", kT), ("v", v),
                                  ("ctxT", ctxT)):
                    nc.gpsimd.dma_start(dbg[name], src[:])
    nc.compile()
    return nc


def _get_nc():
    if "nc" not in _CACHED:
        _CACHED["nc"] = _build()
    return _CACHED["nc"]


def kernel(ppg, ecg, Wq, bq, Wk, bk, Wv, bv, Wo, bo):
    from concourse.bass_utils import run_bass_kernel_spmd

    nc = _get_nc()
    f = np.float32
    wqt = np.ascontiguousarray(np.asarray(Wq, f).T)
    wkt = np.ascontiguousarray(np.asarray(Wk, f).T)
    wvt = np.ascontiguousarray(np.asarray(Wv, f).T)
    wot = np.ascontiguousarray(np.asarray(Wo, f).T)
    ppg = np.asarray(ppg, f)
    ecg = np.asarray(ecg, f)
    in_maps = []
    for c in range(8):
        b, half = c // 2, c % 2
        in_maps.append({
            "ppg_q": np.ascontiguousarray(ppg[b][:, half * LQ:(half + 1) * LQ]),
            "ecg_b": np.ascontiguousarray(ecg[b]),
            "wqt": wqt, "wkt": wkt, "wvt": wvt, "wot": wot,
            "bq": np.asarray(bq, f), "bk": np.asarray(bk, f),
            "bv": np.asarray(bv, f), "bo": np.asarray(bo, f),
        })
    _CACHED["last_in_maps"] = in_maps
    res = run_bass_kernel_spmd(nc, in_maps, core_ids=list(range(8)))
    out = np.empty((B, C, L), f)
    for c, r in enumerate(res.results):
        b, half = c // 2, c % 2
        out[b][:, half * LQ:(half + 1) * LQ] = r["outp"]
    return out
